# revision 9
# baseline (speedup 1.0000x reference)
"""Multi-head attention Trainium2 kernel, 8-core SPMD. v3.

Sharding: 16 (batch, head) pairs over 8 cores -> each core computes 2 heads
of one batch and returns a partial [N, D] output (bf16); host sums 4
partials per batch in fp32.

Per-core dataflow (all layouts transposed, q/m on free dims so softmax's
normalization can be deferred):
  XT = x pre-transposed on host        [D, N] bf16, loaded as [128, DC, N]
                                       (sync HWDGE + gpsimd SWDGE queues)
  QT/KT/VT = W.T @ XT                  [2*HS, N] per head pair (scale folded
                                       into Wq on host)
  Vn = dma-transpose(VT) per (ms,h)    [m 128, mc, h, 65]; col 64 = ones
  S^T[m,q] = KT_h.T @ QT_h             PSUM fp32, per m-chunk of 128
  P^T = exp(S^T)                       ACT, -> SBUF bf16 (no max subtraction:
                                       logits are O(6) by construction)
  O^T[65,q] = [V_h | 1].T @ P^T        PSUM accumulate over m; row 64 = row
                                       sums r[q] (ones-column trick)
  rb = bcast(1/r)                      DVE recip of psum row 64 -> gpsimd
                                       partition_broadcast
  Un = O^T[0:64] * rb                  DVE, psum x sbuf -> sbuf bf16
  out[q,:] += Un_h.T @ Wp_h            both heads stacked on 128 partitions

The (hh, mc) loop is paced by the scalar engine's exp; everything else
(projections, vn transposes, input DMA, normalize, output projection)
trickles through PE/DVE/Pool slack via a deferred-work queue.
"""

import os
import sys

import numpy as np

sys.path.insert(0, "/opt/trn_rl_repo")

import ml_dtypes
from contextlib import ExitStack

import concourse.bass as bass
import concourse.mybir as mybir
import concourse.tile as tile
from concourse import bacc
from concourse.bass_utils import run_bass_kernel_spmd

B, N, D, H, HS = 2, 2048, 512, 8, 64
NCORES = 8
BF16 = mybir.dt.bfloat16
FP32 = mybir.dt.float32
nbf16 = ml_dtypes.bfloat16

DC = D // 128  # 4 d-chunks
MC = N // 128  # 16 m-chunks
MS = N // 512  # 4 m-slices (DMA / proj granularity)
QH = 2  # q halves
QW = N // QH  # 1024 q per chunk


def build_nc(finalize=True):
    nc = bacc.Bacc()
    xqt = nc.dram_tensor("xqt", [D, N], BF16, kind="ExternalInput")
    xkt = nc.dram_tensor("xkt", [D, N], BF16, kind="ExternalInput")
    xvt = nc.dram_tensor("xvt", [D, N], BF16, kind="ExternalInput")
    wq = nc.dram_tensor("wq", [D, 128], BF16, kind="ExternalInput")
    wk = nc.dram_tensor("wk", [D, 128], BF16, kind="ExternalInput")
    wv = nc.dram_tensor("wv", [D, 128], BF16, kind="ExternalInput")
    wp = nc.dram_tensor("wp", [2 * HS, D], BF16, kind="ExternalInput")
    out = nc.dram_tensor("out", [N, D], BF16, kind="ExternalOutput")

    # Manual PSUM bank plan (8 banks x 2KB):
    #   banks 0-1: sA   s_ps for even iterations   [128, 1024] fp32
    #   banks 2-3: sB   s_ps for odd iterations
    #   banks 4-5: oA   O^T accumulator, head 0 (rows 0:65)
    #   banks 6-7: oB   O^T accumulator, head 1; also proj / qh0-outproj
    #              scratch during windows when the h1 accumulator is idle
    sA = nc.alloc_psum_tensor("sA", [128, 1024], FP32)
    sB = nc.alloc_psum_tensor("sB", [128, 1024], FP32)
    oA = nc.alloc_psum_tensor("oA", [128, 1024], FP32)
    oB = nc.alloc_psum_tensor("oB", [128, 1024], FP32)

    with tile.TileContext(nc) as tc, ExitStack() as ctx:
        consts = ctx.enter_context(tc.tile_pool(name="consts", bufs=1))
        xt_pool = ctx.enter_context(tc.tile_pool(name="xt", bufs=1))
        proj_pool = ctx.enter_context(tc.tile_pool(name="proj", bufs=1))
        pt_pool = ctx.enter_context(tc.tile_pool(name="pt", bufs=13))
        un_pool = ctx.enter_context(tc.tile_pool(name="un", bufs=4))
        rr_pool = ctx.enter_context(tc.tile_pool(name="rr", bufs=4))
        rb_pool = ctx.enter_context(tc.tile_pool(name="rb", bufs=4))
        ob_pool = ctx.enter_context(tc.tile_pool(name="ob", bufs=3))
        vst_pool = ctx.enter_context(tc.tile_pool(name="vst", bufs=3))

        wq_s = consts.tile([128, DC, 128], BF16, tag="wq_s")
        wk_s = consts.tile([128, DC, 128], BF16, tag="wk_s")
        wv_s = consts.tile([128, DC, 128], BF16, tag="wv_s")
        wp_s = consts.tile([2 * HS, D], BF16, tag="wp_s")

        xts = {}
        for name in ("q", "k", "v"):
            xts[name] = xt_pool.tile(
                [128, DC, N], BF16, tag=f"xt_{name}", name=f"xt_{name}"
            )

        def load_x(eng, name, ms):
            dram = {"q": xqt, "k": xkt, "v": xvt}[name]
            sl = slice(ms * 512, (ms + 1) * 512)
            eng.dma_start(
                out=xts[name][:, :, sl],
                in_=dram[:, sl].rearrange("(c p) n -> p c n", p=128),
            )

        # sync HWDGE queue: wk, xk0, xq0, xq1, xk1 up front; the rest is
        # trickled through the loop (vn transposes interleave).
        # xq1 rides the scalar engine's own HWDGE queue (idle until the
        # first exp) so xk0/xq0/xq1 all transfer in parallel at t=0.
        # All remaining loads are issued up front too: trickled DMA issues
        # would queue behind vn transposes whose semaphore waits block the
        # sync queue for ~10us on HW.
        load_x(nc.scalar, "q", 1)
        nc.sync.dma_start(out=wk_s[:], in_=wk.rearrange("(c p) h -> p c h", p=128))
        load_x(nc.sync, "k", 0)
        load_x(nc.sync, "k", 1)
        load_x(nc.sync, "k", 2)
        load_x(nc.sync, "k", 3)
        load_x(nc.sync, "q", 2)
        load_x(nc.sync, "q", 3)
        sc_eng = [nc.scalar, nc.vector]  # tail ob-copy engines (alternate)

        # gpsimd SWDGE queue: wv, wq, xq0 (parallel to sync's xk0), xv0..3
        nc.gpsimd.dma_start(
            out=wv_s[:], in_=wv.rearrange("(c p) h -> p c h", p=128)
        )
        nc.gpsimd.dma_start(
            out=wq_s[:], in_=wq.rearrange("(c p) h -> p c h", p=128)
        )
        load_x(nc.gpsimd, "q", 0)
        for ms in range(MS):
            load_x(nc.gpsimd, "v", ms)
        nc.gpsimd.dma_start(out=wp_s[:], in_=wp[:])

        # Vn: [m 128, mc, head, 65]; col 64 = ones (rowsum trick)
        vn = consts.tile([128, MC, 2, HS + 1], BF16, tag="vn")
        nc.gpsimd.memset(vn[:, :, :, HS : HS + 1], 1.0)

        wmap = {"q": wq_s, "k": wk_s, "v": wv_s}
        projT = {}
        for name in ("q", "k", "v"):
            projT[name] = proj_pool.tile(
                [128, N], BF16, tag=f"projT_{name}", name=f"projT_{name}"
            )

        # proj psum lives in oB's banks (the h1 accumulator is idle while
        # projections trickle); alternate halves for copy/matmul overlap
        proj_state = {"slot": 0, "cur": None}

        def emit_proj_half(name, ms, half, copy_eng=None):
            # half 0: dc 0,1 into a psum slot; half 1: dc 2,3 + copy out
            sl = slice(ms * 512, (ms + 1) * 512)
            if half == 0:
                s = proj_state["slot"]
                proj_state["slot"] = 1 - s
                proj_state["cur"] = oB[:, s * 512 : (s + 1) * 512]
            ps = proj_state["cur"]
            for dc in (0, 1) if half == 0 else (2, 3):
                nc.tensor.matmul(
                    ps,
                    wmap[name][:, dc, :],
                    xts[name][:, dc, sl],
                    start=(dc == 0),
                    stop=(dc == DC - 1),
                )
            if half == 1:
                if copy_eng is nc.scalar:
                    nc.scalar.copy(projT[name][:, sl], ps)
                else:
                    nc.vector.tensor_copy(projT[name][:, sl], ps)

        def emit_proj_slice(name, ms, copy_eng=None):
            emit_proj_half(name, ms, 0)
            emit_proj_half(name, ms, 1, copy_eng=copy_eng)

        # The xbar transpose corrupts holey/strided destination APs on HW
        # (fine in sim), so transpose into a contiguous staging tile first
        # and let gpsimd scatter it into vn's [mc, h, 65] layout.
        vstage = {}

        def emit_vnq(ms, h):
            # SBUF->SBUF xbar transpose of one (m-slice, head) quarter of VT
            st = vst_pool.tile([128, 4, HS], BF16, tag="vs", name="vs")
            vstage[(ms, h)] = st
            nc.sync.dma_start(
                out=st[:],
                in_=projT["v"][h * HS : (h + 1) * HS, ms * 512 : (ms + 1) * 512],
                transpose=True,
            )

        def emit_vncp(ms, h):
            nc.gpsimd.tensor_copy(
                vn[:, ms * 4 : (ms + 1) * 4, h, 0:HS], vstage.pop((ms, h))[:]
            )

        # PE p-state warmup: the tensor engine only reaches full clock after
        # ~3us of continuous execution; stream dummy matmuls on a zeroed
        # tile while the first input DMAs are in flight so the real
        # projections start at full speed
        zwarm = consts.tile([128, 128], BF16, tag="zwarm")
        nc.vector.memset(zwarm[:], 0.0)
        for _ in range(24):
            nc.tensor.matmul(
                sA[:, 0:128], zwarm[:], zwarm[:], start=True, stop=True
            )

        # prologue: only what gates iteration 0 (S needs KT slice 0, QT);
        # q0's psum->sbuf copy runs on the still-idle scalar engine so the
        # two q copies don't serialize on DVE ahead of the first S
        emit_proj_slice("k", 0)
        emit_proj_slice("q", 0, copy_eng=nc.scalar)
        emit_proj_slice("q", 1)

        # deferred trickle queue: each item is (pe_cost, [closures]); each
        # iteration consumes items at its TOP until ~2 matmuls' worth of PE
        # work has been queued (non-PE items — DMA issues, Pool copies,
        # normalize stages — are free), so data is always emitted before the
        # iteration that reads it without over-stuffing the PE stream.
        deferred = [
            (2, [lambda: emit_proj_half("v", 0, 0)]),
            (2, [lambda: emit_proj_half("v", 0, 1)]),
            (0, [lambda: emit_vnq(0, 0)]),
            (0, [lambda: emit_vnq(0, 1)]),
            (2, [lambda: emit_proj_half("k", 1, 0)]),
            (2, [lambda: emit_proj_half("k", 1, 1), lambda: emit_vncp(0, 0)]),
            (2, [lambda: emit_proj_half("v", 1, 0)]),
            (2, [lambda: emit_proj_half("v", 1, 1), lambda: emit_vncp(0, 1)]),
            (0, [lambda: emit_vnq(1, 0)]),
            (2, [lambda: emit_vnq(1, 1), lambda: emit_proj_half("k", 2, 0)]),
            (2, [lambda: emit_proj_half("k", 2, 1)]),
            (2, [lambda: emit_proj_half("v", 2, 0)]),
            (2, [lambda: emit_proj_half("v", 2, 1), lambda: emit_vncp(1, 0)]),
            (0, [lambda: emit_vnq(2, 0), lambda: emit_vncp(1, 1)]),
            (2, [lambda: emit_vnq(2, 1), lambda: emit_proj_half("k", 3, 0)]),
            (2, [lambda: emit_proj_half("k", 3, 1), lambda: emit_vncp(2, 0)]),
            (2, [lambda: emit_proj_half("v", 3, 0), lambda: emit_vncp(2, 1)]),
            (2, [lambda: emit_proj_half("v", 3, 1)]),
            (0, [lambda: emit_vnq(3, 0)]),
            (2, [lambda: emit_vnq(3, 1), lambda: emit_proj_half("q", 2, 0)]),
            (2, [lambda: emit_proj_half("q", 2, 1), lambda: emit_vncp(3, 0)]),
            (2, [lambda: emit_proj_half("q", 3, 0), lambda: emit_vncp(3, 1)]),
            (2, [lambda: emit_proj_half("q", 3, 1)]),
        ]

        # normalization chain, split into trickle-able stages.
        # rr = 1/rowsum (DVE, psum row 64 -> sbuf fp32 at partition 0)
        # rb = broadcast to 64 partitions (gpsimd)
        # un = O^T[0:64] * rb (DVE, psum x sbuf -> sbuf bf16)
        def emit_recip(o_ps_, hh, q0, q1):
            # reciprocal_approx_fast corrupts on HW when its input is a PSUM
            # row off partition 0; copy the rowsum row down first (DVE
            # partition-shift on a plain TensorCopy is exact)
            r0 = rr_pool.tile([1, QW], FP32, tag="r0", name="r0")
            nc.vector.tensor_copy(r0[0:1, q0:q1], o_ps_[hh][HS : HS + 1, q0:q1])
            rr = rr_pool.tile([1, QW], FP32, tag="rr", name="rr")
            nc.vector.reciprocal_approx_fast(rr[0:1, q0:q1], r0[0:1, q0:q1])
            return rr

        def emit_bcast(rr, q0, q1):
            rb = rb_pool.tile([HS, QW], FP32, tag="rb", name="rb")
            nc.gpsimd.partition_broadcast(rb[:, q0:q1], rr[0:1, q0:q1])
            return rb

        def emit_mul(o_ps_, rb, un2_, hh, q0, q1, eng=None):
            (eng or nc.vector).tensor_mul(
                un2_[HS * hh : HS * hh + HS, q0:q1],
                o_ps_[hh][0:HS, q0:q1],
                rb[:, q0:q1],
            )

        def emit_final(qh_, un2_, c, f_ps, ob_eng=None):
            nc.tensor.matmul(
                f_ps,
                un2_[:, c * 128 : (c + 1) * 128],
                wp_s[:],
                start=True,
                stop=True,
            )
            ob = ob_pool.tile([128, D], BF16, tag="ob", name="ob")
            if ob_eng is nc.scalar:
                nc.scalar.copy(ob[:], f_ps)
            else:
                (ob_eng or nc.vector).tensor_copy(ob[:], f_ps)
            nc.sync.dma_start(
                out=out[qh_ * QW + c * 128 : qh_ * QW + (c + 1) * 128, :],
                in_=ob[:],
            )

        # attention loop: heads sequential within each q-half so h0's
        # normalization trickles during h1's m-loop; leftover PVs at a qh
        # boundary drain into the next qh's loop instead of bursting
        qt2, kt2 = projT["q"], projT["k"]
        o_ps = {0: oA, 1: oB}
        s_it = [0]  # global S iteration counter for sA/sB alternation
        pend = []

        def pv(hh, j, p_sb):
            for sl in range(QW // 512):
                nc.tensor.matmul(
                    o_ps[hh][0 : HS + 1, sl * 512 : (sl + 1) * 512],
                    vn[:, j, hh, :],
                    p_sb[:, sl * 512 : (sl + 1) * 512],
                    start=(j == 0),
                    stop=(j == MC - 1),
                )

        def chain_items(hh, un2_):
            # normalization chain for one completed head, as trickle items
            box = {}
            return [
                (0, [lambda: box.__setitem__("rr", emit_recip(o_ps, hh, 0, QW))]),
                (0, [lambda: box.__setitem__("rb", emit_bcast(box["rr"], 0, QW))]),
                (0, [lambda: emit_mul(o_ps, box["rb"], un2_, hh, 0, QW)]),
            ]

        def pv_pop():
            qh_, hh_, mc_, p_, un2_ = pend.pop(0)
            pv(hh_, mc_, p_)
            if mc_ == MC - 1:
                if hh_ == 0:
                    deferred.extend(chain_items(0, un2_))
                elif qh_ < QH - 1:
                    deferred.extend(chain_items(1, un2_))
                    for c in range(QW // 128):
                        # f_ps alternates oB halves (h1 accumulator idle)
                        deferred.append(
                            (
                                2,
                                [
                                    lambda un2__=un2_, c_=c, qh__=qh_: emit_final(
                                        qh__,
                                        un2__,
                                        c_,
                                        oB[
                                            :,
                                            (c_ % 2) * 512 : (c_ % 2) * 512
                                            + 512,
                                        ],
                                    )
                                ],
                            )
                        )

        for qh in range(QH):
            un2 = un_pool.tile([128, QW], BF16, tag="un")
            for hh in range(2):
                for mc in range(MC):
                    it = hh * MC + mc
                    budget = 2 if (qh == 0 and it < 12) else 1
                    nb = 0
                    while deferred and nb < budget:
                        for fn in deferred.pop(0)[1]:
                            fn()
                        nb += 1
                    hs0 = HS * hh
                    s_ps = (sA, sB)[s_it[0] % 2]
                    s_it[0] += 1
                    for sl in range(QW // 512):
                        nc.tensor.matmul(
                            s_ps[:, sl * 512 : (sl + 1) * 512],
                            kt2[hs0 : hs0 + HS, mc * 128 : (mc + 1) * 128],
                            qt2[
                                hs0 : hs0 + HS,
                                qh * QW + sl * 512 : qh * QW + (sl + 1) * 512,
                            ],
                            start=True,
                            stop=True,
                        )
                    p_sb = pt_pool.tile([128, QW], BF16, tag="p", name="p_sb")
                    last = qh == QH - 1 and it == 2 * MC - 1
                    if last:
                        # split the final exp so PV / normalize / outproj of
                        # the first q-half start half an exp earlier
                        for sl in range(2):
                            nc.scalar.activation(
                                p_sb[:, sl * 512 : (sl + 1) * 512],
                                s_ps[:, sl * 512 : (sl + 1) * 512],
                                mybir.ActivationFunctionType.Exp,
                            )
                    else:
                        nc.scalar.activation(
                            p_sb[:], s_ps[:], mybir.ActivationFunctionType.Exp
                        )
                    pend.append((qh, hh, mc, p_sb, un2))
                    # qh0: lag 8 (vn staging arrives ~iter 10); qh1: lag 5
                    # so the previous qh's leftovers (and with them the
                    # trickled outproj into oB) clear before PV(h1) reopens oB
                    lag = (8 if qh == 0 else 5) if not last else 1
                    nd = 0
                    while len(pend) > lag and nd < 2:
                        pv_pop()
                        nd += 1
        while pend:
            pv_pop()
        # flush any remaining trickle items (qh1 h0-chain may still be queued)
        while deferred:
            for fn in deferred.pop(0)[1]:
                fn()

        # tail: last head's chain split by q-halves; outproj pipelined
        # behind on four f_ps slots in the freed S banks, ob copies
        # alternating scalar/vector (both idle by now)
        rr0 = emit_recip(o_ps, 1, 0, 512)
        rb0 = emit_bcast(rr0, 0, 512)
        emit_mul(o_ps, rb0, un2, 1, 0, 512)
        rr1 = emit_recip(o_ps, 1, 512, QW)
        rb1 = emit_bcast(rr1, 512, QW)
        emit_mul(o_ps, rb1, un2, 1, 512, QW)
        f_slots = [sA[:, 0:512], sA[:, 512:1024], sB[:, 0:512], sB[:, 512:1024]]
        for c in range(QW // 128):
            emit_final(QH - 1, un2, c, f_slots[c % 4], ob_eng=sc_eng[c % 2])
    if finalize:
        nc.finalize()
    return nc


_NC_CACHE = None


def _get_nc():
    global _NC_CACHE
    if _NC_CACHE is None:
        _NC_CACHE = build_nc()
    return _NC_CACHE


def make_in_maps(inputs):
    query = np.asarray(inputs["query"], np.float32)
    key = np.asarray(inputs["key"], np.float32)
    value = np.asarray(inputs["value"], np.float32)
    Wq = np.asarray(inputs["Wq"], np.float32) / np.sqrt(np.float32(HS))
    Wk = np.asarray(inputs["Wk"], np.float32)
    Wv = np.asarray(inputs["Wv"], np.float32)
    Wp = np.asarray(inputs["Wp"], np.float32)

    in_maps = []
    for c in range(NCORES):
        b = c // 4
        h0 = 2 * (c % 4)
        in_maps.append(
            {
                "xqt": np.ascontiguousarray(query[b].T).astype(nbf16),
                "xkt": np.ascontiguousarray(key[b].T).astype(nbf16),
                "xvt": np.ascontiguousarray(value[b].T).astype(nbf16),
                "wq": np.concatenate([Wq[h0], Wq[h0 + 1]], axis=1).astype(nbf16),
                "wk": np.concatenate([Wk[h0], Wk[h0 + 1]], axis=1).astype(nbf16),
                "wv": np.concatenate([Wv[h0], Wv[h0 + 1]], axis=1).astype(nbf16),
                "wp": np.concatenate([Wp[h0], Wp[h0 + 1]], axis=0).astype(nbf16),
            }
        )
    return in_maps


def kernel(query, key, value, Wq, Wk, Wv, Wp):
    in_maps = make_in_maps(
        dict(query=query, key=key, value=value, Wq=Wq, Wk=Wk, Wv=Wv, Wp=Wp)
    )
    nc = _get_nc()
    res = run_bass_kernel_spmd(nc, in_maps, list(range(NCORES)))
    out = np.zeros((B, N, D), np.float32)
    for c in range(NCORES):
        out[c // 4] += np.asarray(res.results[c]["out"], np.float32)
    return out


if __name__ == "__main__":
    d = np.load("/root/problem/work/ref.npz")
    got = kernel(
        d["query"], d["key"], d["value"], d["Wq"], d["Wk"], d["Wv"], d["Wp"]
    )
    exp = d["expected"]
    rel = np.linalg.norm(got - exp) / np.linalg.norm(exp)
    print("Relative error:", rel)


# revision 10
# speedup vs baseline: 1.1480x; 1.1480x over previous
"""Multi-head attention Trainium2 kernel, 8-core SPMD. v3.

Sharding: 16 (batch, head) pairs over 8 cores -> each core computes 2 heads
of one batch and returns a partial [N, D] output (bf16); host sums 4
partials per batch in fp32.

Per-core dataflow (all layouts transposed, q/m on free dims so softmax's
normalization can be deferred):
  XT = x pre-transposed on host        [D, N] bf16, loaded as [128, DC, N]
                                       (sync HWDGE + gpsimd SWDGE queues)
  QT/KT/VT = W.T @ XT                  [2*HS, N] per head pair (scale folded
                                       into Wq on host)
  Vn = dma-transpose(VT) per (ms,h)    [m 128, mc, h, 65]; col 64 = ones
  S^T[m,q] = KT_h.T @ QT_h             PSUM fp32, per m-chunk of 128
  P^T = exp(S^T)                       ACT, -> SBUF bf16 (no max subtraction:
                                       logits are O(6) by construction)
  O^T[65,q] = [V_h | 1].T @ P^T        PSUM accumulate over m; row 64 = row
                                       sums r[q] (ones-column trick)
  rb = bcast(1/r)                      DVE recip of psum row 64 -> gpsimd
                                       partition_broadcast
  Un = O^T[0:64] * rb                  DVE, psum x sbuf -> sbuf bf16
  out[q,:] += Un_h.T @ Wp_h            both heads stacked on 128 partitions

The (hh, mc) loop is paced by the scalar engine's exp; everything else
(projections, vn transposes, input DMA, normalize, output projection)
trickles through PE/DVE/Pool slack via a deferred-work queue.
"""

import os
import sys

import numpy as np

sys.path.insert(0, "/opt/trn_rl_repo")

import ml_dtypes
from contextlib import ExitStack

import concourse.bass as bass
import concourse.mybir as mybir
import concourse.tile as tile
from concourse import bacc
from concourse.bass_utils import run_bass_kernel_spmd

B, N, D, H, HS = 2, 2048, 512, 8, 64
NCORES = 8
BF16 = mybir.dt.bfloat16
FP32 = mybir.dt.float32
nbf16 = ml_dtypes.bfloat16

DC = D // 128  # 4 d-chunks
MC = N // 128  # 16 m-chunks
MS = N // 512  # 4 m-slices (DMA / proj granularity)
QH = 2  # q halves
QW = N // QH  # 1024 q per chunk


def build_nc(finalize=True):
    nc = bacc.Bacc()
    xqt = nc.dram_tensor("xqt", [D, N], BF16, kind="ExternalInput")
    xkt = nc.dram_tensor("xkt", [D, N], BF16, kind="ExternalInput")
    xvt = nc.dram_tensor("xvt", [D, N], BF16, kind="ExternalInput")
    wq = nc.dram_tensor("wq", [D, 128], BF16, kind="ExternalInput")
    wk = nc.dram_tensor("wk", [D, 128], BF16, kind="ExternalInput")
    wv = nc.dram_tensor("wv", [D, 128], BF16, kind="ExternalInput")
    wp = nc.dram_tensor("wp", [2 * HS, D], BF16, kind="ExternalInput")
    out = nc.dram_tensor("out", [N, D], BF16, kind="ExternalOutput")

    # Manual PSUM bank plan (8 banks x 2KB):
    #   banks 0-1: sA   s_ps for even iterations   [128, 1024] fp32
    #   banks 2-3: sB   s_ps for odd iterations
    #   banks 4-5: oA   O^T accumulator, head 0 (rows 0:65)
    #   banks 6-7: oB   O^T accumulator, head 1; also proj / qh0-outproj
    #              scratch during windows when the h1 accumulator is idle
    sA = nc.alloc_psum_tensor("sA", [128, 1024], FP32)
    sB = nc.alloc_psum_tensor("sB", [128, 1024], FP32)
    oA = nc.alloc_psum_tensor("oA", [128, 1024], FP32)
    oB = nc.alloc_psum_tensor("oB", [128, 1024], FP32)

    with tile.TileContext(nc) as tc, ExitStack() as ctx:
        consts = ctx.enter_context(tc.tile_pool(name="consts", bufs=1))
        xt_pool = ctx.enter_context(tc.tile_pool(name="xt", bufs=1))
        proj_pool = ctx.enter_context(tc.tile_pool(name="proj", bufs=1))
        pt_pool = ctx.enter_context(tc.tile_pool(name="pt", bufs=13))
        un_pool = ctx.enter_context(tc.tile_pool(name="un", bufs=4))
        rr_pool = ctx.enter_context(tc.tile_pool(name="rr", bufs=4))
        rb_pool = ctx.enter_context(tc.tile_pool(name="rb", bufs=4))
        ob_pool = ctx.enter_context(tc.tile_pool(name="ob", bufs=3))
        vst_pool = ctx.enter_context(tc.tile_pool(name="vst", bufs=3))

        wq_s = consts.tile([128, DC, 128], BF16, tag="wq_s")
        wk_s = consts.tile([128, DC, 128], BF16, tag="wk_s")
        wv_s = consts.tile([128, DC, 128], BF16, tag="wv_s")
        wp_s = consts.tile([2 * HS, D], BF16, tag="wp_s")

        xts = {}
        for name in ("q", "k", "v"):
            xts[name] = xt_pool.tile(
                [128, DC, N], BF16, tag=f"xt_{name}", name=f"xt_{name}"
            )

        def load_x(eng, name, ms):
            dram = {"q": xqt, "k": xkt, "v": xvt}[name]
            sl = slice(ms * 512, (ms + 1) * 512)
            eng.dma_start(
                out=xts[name][:, :, sl],
                in_=dram[:, sl].rearrange("(c p) n -> p c n", p=128),
            )

        # sync HWDGE queue: wk, xk0, xq0, xq1, xk1 up front; the rest is
        # trickled through the loop (vn transposes interleave).
        # xq1 rides the scalar engine's own HWDGE queue (idle until the
        # first exp) so xk0/xq0/xq1 all transfer in parallel at t=0.
        # All remaining loads are issued up front too: trickled DMA issues
        # would queue behind vn transposes whose semaphore waits block the
        # sync queue for ~10us on HW.
        load_x(nc.scalar, "q", 1)
        nc.sync.dma_start(out=wk_s[:], in_=wk.rearrange("(c p) h -> p c h", p=128))
        load_x(nc.sync, "k", 0)
        load_x(nc.sync, "k", 1)
        load_x(nc.sync, "k", 2)
        load_x(nc.sync, "k", 3)
        load_x(nc.sync, "q", 2)
        load_x(nc.sync, "q", 3)
        sc_eng = [nc.scalar, nc.vector]  # tail ob-copy engines (alternate)

        # gpsimd SWDGE queue: wv, wq, xq0 (parallel to sync's xk0), xv0..3
        nc.gpsimd.dma_start(
            out=wv_s[:], in_=wv.rearrange("(c p) h -> p c h", p=128)
        )
        nc.gpsimd.dma_start(
            out=wq_s[:], in_=wq.rearrange("(c p) h -> p c h", p=128)
        )
        load_x(nc.gpsimd, "q", 0)
        for ms in range(MS):
            load_x(nc.gpsimd, "v", ms)
        nc.gpsimd.dma_start(out=wp_s[:], in_=wp[:])

        # Vn: [m 128, mc, head, 65]; col 64 = ones (rowsum trick)
        vn = consts.tile([128, MC, 2, HS + 1], BF16, tag="vn")
        nc.gpsimd.memset(vn[:, :, :, HS : HS + 1], 1.0)

        wmap = {"q": wq_s, "k": wk_s, "v": wv_s}
        projT = {}
        for name in ("q", "k", "v"):
            projT[name] = proj_pool.tile(
                [128, N], BF16, tag=f"projT_{name}", name=f"projT_{name}"
            )

        # proj psum lives in oB's banks (the h1 accumulator is idle while
        # projections trickle); alternate halves for copy/matmul overlap
        proj_state = {"slot": 0, "cur": None}

        def emit_proj_half(name, ms, half, copy_eng=None):
            # half 0: dc 0,1 into a psum slot; half 1: dc 2,3 + copy out
            sl = slice(ms * 512, (ms + 1) * 512)
            if half == 0:
                s = proj_state["slot"]
                proj_state["slot"] = 1 - s
                proj_state["cur"] = oB[:, s * 512 : (s + 1) * 512]
            ps = proj_state["cur"]
            for dc in (0, 1) if half == 0 else (2, 3):
                nc.tensor.matmul(
                    ps,
                    wmap[name][:, dc, :],
                    xts[name][:, dc, sl],
                    start=(dc == 0),
                    stop=(dc == DC - 1),
                )
            if half == 1:
                if copy_eng is nc.scalar:
                    nc.scalar.copy(projT[name][:, sl], ps)
                else:
                    nc.vector.tensor_copy(projT[name][:, sl], ps)

        def emit_proj_slice(name, ms, copy_eng=None):
            emit_proj_half(name, ms, 0)
            emit_proj_half(name, ms, 1, copy_eng=copy_eng)

        # The xbar transpose corrupts holey/strided destination APs on HW
        # (fine in sim), so transpose into a contiguous staging tile first
        # and let gpsimd scatter it into vn's [mc, h, 65] layout.
        vstage = {}

        def emit_vnq(ms, h):
            # SBUF->SBUF xbar transpose of one (m-slice, head) quarter of VT
            st = vst_pool.tile([128, 4, HS], BF16, tag="vs", name="vs")
            vstage[(ms, h)] = st
            nc.sync.dma_start(
                out=st[:],
                in_=projT["v"][h * HS : (h + 1) * HS, ms * 512 : (ms + 1) * 512],
                transpose=True,
            )

        def emit_vncp(ms, h):
            nc.gpsimd.tensor_copy(
                vn[:, ms * 4 : (ms + 1) * 4, h, 0:HS], vstage.pop((ms, h))[:]
            )

        # PE p-state warmup: the tensor engine only reaches full clock after
        # ~3us of continuous execution; stream dummy matmuls on a zeroed
        # tile while the first input DMAs are in flight so the real
        # projections start at full speed
        # (memset on gpsimd: it lands after gpsimd's DMA issues, which
        # delays the warmup to right before the first real projection —
        # ending warmup early would let the PE clock drop again)
        zwarm = consts.tile([128, 128], BF16, tag="zwarm")
        nc.gpsimd.memset(zwarm[:], 0.0)
        for _ in range(24):
            nc.tensor.matmul(
                sA[:, 0:128], zwarm[:], zwarm[:], start=True, stop=True
            )

        # prologue: only what gates iteration 0 (S needs KT slice 0, QT);
        # q0's psum->sbuf copy runs on the still-idle scalar engine so the
        # two q copies don't serialize on DVE ahead of the first S
        emit_proj_slice("k", 0)
        emit_proj_slice("q", 0, copy_eng=nc.scalar)
        emit_proj_slice("q", 1)

        # deferred trickle queue: each item is (pe_cost, [closures]); each
        # iteration consumes items at its TOP until ~2 matmuls' worth of PE
        # work has been queued (non-PE items — DMA issues, Pool copies,
        # normalize stages — are free), so data is always emitted before the
        # iteration that reads it without over-stuffing the PE stream.
        deferred = [
            (2, [lambda: emit_proj_half("v", 0, 0)]),
            (2, [lambda: emit_proj_half("v", 0, 1)]),
            (0, [lambda: emit_vnq(0, 0)]),
            (0, [lambda: emit_vnq(0, 1)]),
            (2, [lambda: emit_proj_half("k", 1, 0)]),
            (2, [lambda: emit_proj_half("k", 1, 1), lambda: emit_vncp(0, 0)]),
            (2, [lambda: emit_proj_half("v", 1, 0)]),
            (2, [lambda: emit_proj_half("v", 1, 1), lambda: emit_vncp(0, 1)]),
            (0, [lambda: emit_vnq(1, 0)]),
            (2, [lambda: emit_vnq(1, 1), lambda: emit_proj_half("k", 2, 0)]),
            (2, [lambda: emit_proj_half("k", 2, 1)]),
            (2, [lambda: emit_proj_half("v", 2, 0)]),
            (2, [lambda: emit_proj_half("v", 2, 1), lambda: emit_vncp(1, 0)]),
            (0, [lambda: emit_vnq(2, 0), lambda: emit_vncp(1, 1)]),
            (2, [lambda: emit_vnq(2, 1), lambda: emit_proj_half("k", 3, 0)]),
            (2, [lambda: emit_proj_half("k", 3, 1), lambda: emit_vncp(2, 0)]),
            (2, [lambda: emit_proj_half("v", 3, 0), lambda: emit_vncp(2, 1)]),
            (2, [lambda: emit_proj_half("v", 3, 1)]),
            (0, [lambda: emit_vnq(3, 0)]),
            (2, [lambda: emit_vnq(3, 1), lambda: emit_proj_half("q", 2, 0)]),
            (2, [lambda: emit_proj_half("q", 2, 1), lambda: emit_vncp(3, 0)]),
            (2, [lambda: emit_proj_half("q", 3, 0), lambda: emit_vncp(3, 1)]),
            (2, [lambda: emit_proj_half("q", 3, 1)]),
        ]

        # normalization chain, split into trickle-able stages.
        # rr = 1/rowsum (DVE, psum row 64 -> sbuf fp32 at partition 0)
        # rb = broadcast to 64 partitions (gpsimd)
        # un = O^T[0:64] * rb (DVE, psum x sbuf -> sbuf bf16)
        def emit_recip(o_ps_, hh, q0, q1):
            # reciprocal_approx_fast corrupts on HW when its input is a PSUM
            # row off partition 0; copy the rowsum row down first (DVE
            # partition-shift on a plain TensorCopy is exact)
            r0 = rr_pool.tile([1, QW], FP32, tag="r0", name="r0")
            nc.vector.tensor_copy(r0[0:1, q0:q1], o_ps_[hh][HS : HS + 1, q0:q1])
            rr = rr_pool.tile([1, QW], FP32, tag="rr", name="rr")
            nc.vector.reciprocal_approx_fast(rr[0:1, q0:q1], r0[0:1, q0:q1])
            return rr

        def emit_bcast(rr, q0, q1):
            rb = rb_pool.tile([HS, QW], FP32, tag="rb", name="rb")
            nc.gpsimd.partition_broadcast(rb[:, q0:q1], rr[0:1, q0:q1])
            return rb

        def emit_mul(o_ps_, rb, un2_, hh, q0, q1, eng=None):
            (eng or nc.vector).tensor_mul(
                un2_[HS * hh : HS * hh + HS, q0:q1],
                o_ps_[hh][0:HS, q0:q1],
                rb[:, q0:q1],
            )

        def emit_final(qh_, un2_, c, f_ps, ob_eng=None):
            nc.tensor.matmul(
                f_ps,
                un2_[:, c * 128 : (c + 1) * 128],
                wp_s[:],
                start=True,
                stop=True,
            )
            ob = ob_pool.tile([128, D], BF16, tag="ob", name="ob")
            if ob_eng is nc.scalar:
                nc.scalar.copy(ob[:], f_ps)
            else:
                (ob_eng or nc.vector).tensor_copy(ob[:], f_ps)
            nc.sync.dma_start(
                out=out[qh_ * QW + c * 128 : qh_ * QW + (c + 1) * 128, :],
                in_=ob[:],
            )

        # attention loop: heads sequential within each q-half so h0's
        # normalization trickles during h1's m-loop; leftover PVs at a qh
        # boundary drain into the next qh's loop instead of bursting
        qt2, kt2 = projT["q"], projT["k"]
        o_ps = {0: oA, 1: oB}
        s_it = [0]  # global S iteration counter for sA/sB alternation
        pend = []

        def pv(hh, j, p_sb):
            for sl in range(QW // 512):
                nc.tensor.matmul(
                    o_ps[hh][0 : HS + 1, sl * 512 : (sl + 1) * 512],
                    vn[:, j, hh, :],
                    p_sb[:, sl * 512 : (sl + 1) * 512],
                    start=(j == 0),
                    stop=(j == MC - 1),
                )

        def chain_items(hh, un2_):
            # normalization chain for one completed head, as trickle items
            box = {}
            return [
                (0, [lambda: box.__setitem__("rr", emit_recip(o_ps, hh, 0, QW))]),
                (0, [lambda: box.__setitem__("rb", emit_bcast(box["rr"], 0, QW))]),
                (0, [lambda: emit_mul(o_ps, box["rb"], un2_, hh, 0, QW)]),
            ]

        def pv_pop():
            qh_, hh_, mc_, p_, un2_ = pend.pop(0)
            pv(hh_, mc_, p_)
            if mc_ == MC - 1:
                if hh_ == 0:
                    deferred.extend(chain_items(0, un2_))
                elif qh_ < QH - 1:
                    deferred.extend(chain_items(1, un2_))
                    for c in range(QW // 128):
                        # f_ps alternates oB halves (h1 accumulator idle)
                        deferred.append(
                            (
                                2,
                                [
                                    lambda un2__=un2_, c_=c, qh__=qh_: emit_final(
                                        qh__,
                                        un2__,
                                        c_,
                                        oB[
                                            :,
                                            (c_ % 2) * 512 : (c_ % 2) * 512
                                            + 512,
                                        ],
                                    )
                                ],
                            )
                        )

        for qh in range(QH):
            un2 = un_pool.tile([128, QW], BF16, tag="un")
            for hh in range(2):
                for mc in range(MC):
                    it = hh * MC + mc
                    budget = 2 if (qh == 0 and it < 12) else 1
                    nb = 0
                    while deferred and nb < budget:
                        for fn in deferred.pop(0)[1]:
                            fn()
                        nb += 1
                    hs0 = HS * hh
                    s_ps = (sA, sB)[s_it[0] % 2]
                    s_it[0] += 1
                    for sl in range(QW // 512):
                        nc.tensor.matmul(
                            s_ps[:, sl * 512 : (sl + 1) * 512],
                            kt2[hs0 : hs0 + HS, mc * 128 : (mc + 1) * 128],
                            qt2[
                                hs0 : hs0 + HS,
                                qh * QW + sl * 512 : qh * QW + (sl + 1) * 512,
                            ],
                            start=True,
                            stop=True,
                        )
                    p_sb = pt_pool.tile([128, QW], BF16, tag="p", name="p_sb")
                    last = qh == QH - 1 and it == 2 * MC - 1
                    if last:
                        # split the final exp so PV / normalize / outproj of
                        # the first q-half start half an exp earlier
                        for sl in range(2):
                            nc.scalar.activation(
                                p_sb[:, sl * 512 : (sl + 1) * 512],
                                s_ps[:, sl * 512 : (sl + 1) * 512],
                                mybir.ActivationFunctionType.Exp,
                            )
                    else:
                        nc.scalar.activation(
                            p_sb[:], s_ps[:], mybir.ActivationFunctionType.Exp
                        )
                    pend.append((qh, hh, mc, p_sb, un2))
                    # qh0: lag 8 (vn staging arrives ~iter 10); qh1: lag 5
                    # so the previous qh's leftovers (and with them the
                    # trickled outproj into oB) clear before PV(h1) reopens oB
                    lag = (8 if qh == 0 else 5) if not last else 1
                    nd = 0
                    while len(pend) > lag and nd < 2:
                        pv_pop()
                        nd += 1
        while pend:
            pv_pop()
        # flush any remaining trickle items (qh1 h0-chain may still be queued)
        while deferred:
            for fn in deferred.pop(0)[1]:
                fn()

        # tail: last head's chain split by q-halves; outproj pipelined
        # behind on four f_ps slots in the freed S banks, ob copies
        # alternating scalar/vector (both idle by now)
        rr0 = emit_recip(o_ps, 1, 0, 512)
        rb0 = emit_bcast(rr0, 0, 512)
        emit_mul(o_ps, rb0, un2, 1, 0, 512)
        rr1 = emit_recip(o_ps, 1, 512, QW)
        rb1 = emit_bcast(rr1, 512, QW)
        emit_mul(o_ps, rb1, un2, 1, 512, QW)
        f_slots = [sA[:, 0:512], sA[:, 512:1024], sB[:, 0:512], sB[:, 512:1024]]
        for c in range(QW // 128):
            emit_final(QH - 1, un2, c, f_slots[c % 4], ob_eng=sc_eng[c % 2])
    if finalize:
        nc.finalize()
    return nc


_NC_CACHE = None


def _get_nc():
    global _NC_CACHE
    if _NC_CACHE is None:
        _NC_CACHE = build_nc()
    return _NC_CACHE


def make_in_maps(inputs):
    query = np.asarray(inputs["query"], np.float32)
    key = np.asarray(inputs["key"], np.float32)
    value = np.asarray(inputs["value"], np.float32)
    Wq = np.asarray(inputs["Wq"], np.float32) / np.sqrt(np.float32(HS))
    Wk = np.asarray(inputs["Wk"], np.float32)
    Wv = np.asarray(inputs["Wv"], np.float32)
    Wp = np.asarray(inputs["Wp"], np.float32)

    in_maps = []
    for c in range(NCORES):
        b = c // 4
        h0 = 2 * (c % 4)
        in_maps.append(
            {
                "xqt": np.ascontiguousarray(query[b].T).astype(nbf16),
                "xkt": np.ascontiguousarray(key[b].T).astype(nbf16),
                "xvt": np.ascontiguousarray(value[b].T).astype(nbf16),
                "wq": np.concatenate([Wq[h0], Wq[h0 + 1]], axis=1).astype(nbf16),
                "wk": np.concatenate([Wk[h0], Wk[h0 + 1]], axis=1).astype(nbf16),
                "wv": np.concatenate([Wv[h0], Wv[h0 + 1]], axis=1).astype(nbf16),
                "wp": np.concatenate([Wp[h0], Wp[h0 + 1]], axis=0).astype(nbf16),
            }
        )
    return in_maps


def kernel(query, key, value, Wq, Wk, Wv, Wp):
    in_maps = make_in_maps(
        dict(query=query, key=key, value=value, Wq=Wq, Wk=Wk, Wv=Wv, Wp=Wp)
    )
    nc = _get_nc()
    res = run_bass_kernel_spmd(nc, in_maps, list(range(NCORES)))
    out = np.zeros((B, N, D), np.float32)
    for c in range(NCORES):
        out[c // 4] += np.asarray(res.results[c]["out"], np.float32)
    return out


if __name__ == "__main__":
    d = np.load("/root/problem/work/ref.npz")
    got = kernel(
        d["query"], d["key"], d["value"], d["Wq"], d["Wk"], d["Wv"], d["Wp"]
    )
    exp = d["expected"]
    rel = np.linalg.norm(got - exp) / np.linalg.norm(exp)
    print("Relative error:", rel)


# revision 11
# speedup vs baseline: 1.1556x; 1.0066x over previous
"""Multi-head attention Trainium2 kernel, 8-core SPMD. v3.

Sharding: 16 (batch, head) pairs over 8 cores -> each core computes 2 heads
of one batch and returns a partial [N, D] output (bf16); host sums 4
partials per batch in fp32.

Per-core dataflow (all layouts transposed, q/m on free dims so softmax's
normalization can be deferred):
  XT = x pre-transposed on host        [D, N] bf16, loaded as [128, DC, N]
                                       (sync HWDGE + gpsimd SWDGE queues)
  QT/KT/VT = W.T @ XT                  [2*HS, N] per head pair (scale folded
                                       into Wq on host)
  Vn = dma-transpose(VT) per (ms,h)    [m 128, mc, h, 65]; col 64 = ones
  S^T[m,q] = KT_h.T @ QT_h             PSUM fp32, per m-chunk of 128
  P^T = exp(S^T)                       ACT, -> SBUF bf16 (no max subtraction:
                                       logits are O(6) by construction)
  O^T[65,q] = [V_h | 1].T @ P^T        PSUM accumulate over m; row 64 = row
                                       sums r[q] (ones-column trick)
  rb = bcast(1/r)                      DVE recip of psum row 64 -> gpsimd
                                       partition_broadcast
  Un = O^T[0:64] * rb                  DVE, psum x sbuf -> sbuf bf16
  out[q,:] += Un_h.T @ Wp_h            both heads stacked on 128 partitions

The (hh, mc) loop is paced by the scalar engine's exp; everything else
(projections, vn transposes, input DMA, normalize, output projection)
trickles through PE/DVE/Pool slack via a deferred-work queue.
"""

import os
import sys

import numpy as np

sys.path.insert(0, "/opt/trn_rl_repo")

import ml_dtypes
from contextlib import ExitStack

import concourse.bass as bass
import concourse.mybir as mybir
import concourse.tile as tile
from concourse import bacc
from concourse.bass_utils import run_bass_kernel_spmd

B, N, D, H, HS = 2, 2048, 512, 8, 64
NCORES = 8
BF16 = mybir.dt.bfloat16
FP32 = mybir.dt.float32
nbf16 = ml_dtypes.bfloat16

DC = D // 128  # 4 d-chunks
MC = N // 128  # 16 m-chunks
MS = N // 512  # 4 m-slices (DMA / proj granularity)
QH = 2  # q halves
QW = N // QH  # 1024 q per chunk


def build_nc(finalize=True):
    nc = bacc.Bacc()
    xqt = nc.dram_tensor("xqt", [D, N], BF16, kind="ExternalInput")
    xkt = nc.dram_tensor("xkt", [D, N], BF16, kind="ExternalInput")
    xvt = nc.dram_tensor("xvt", [D, N], BF16, kind="ExternalInput")
    wq = nc.dram_tensor("wq", [D, 128], BF16, kind="ExternalInput")
    wk = nc.dram_tensor("wk", [D, 128], BF16, kind="ExternalInput")
    wv = nc.dram_tensor("wv", [D, 128], BF16, kind="ExternalInput")
    wp = nc.dram_tensor("wp", [2 * HS, D], BF16, kind="ExternalInput")
    out = nc.dram_tensor("out", [N, D], BF16, kind="ExternalOutput")

    # Manual PSUM bank plan (8 banks x 2KB):
    #   banks 0-1: sA   s_ps for even iterations   [128, 1024] fp32
    #   banks 2-3: sB   s_ps for odd iterations
    #   banks 4-5: oA   O^T accumulator, head 0 (rows 0:65)
    #   banks 6-7: oB   O^T accumulator, head 1; also proj / qh0-outproj
    #              scratch during windows when the h1 accumulator is idle
    sA = nc.alloc_psum_tensor("sA", [128, 1024], FP32)
    sB = nc.alloc_psum_tensor("sB", [128, 1024], FP32)
    oA = nc.alloc_psum_tensor("oA", [128, 1024], FP32)
    oB = nc.alloc_psum_tensor("oB", [128, 1024], FP32)

    with tile.TileContext(nc) as tc, ExitStack() as ctx:
        consts = ctx.enter_context(tc.tile_pool(name="consts", bufs=1))
        xt_pool = ctx.enter_context(tc.tile_pool(name="xt", bufs=1))
        proj_pool = ctx.enter_context(tc.tile_pool(name="proj", bufs=1))
        pt_pool = ctx.enter_context(tc.tile_pool(name="pt", bufs=13))
        un_pool = ctx.enter_context(tc.tile_pool(name="un", bufs=4))
        rr_pool = ctx.enter_context(tc.tile_pool(name="rr", bufs=4))
        rb_pool = ctx.enter_context(tc.tile_pool(name="rb", bufs=4))
        ob_pool = ctx.enter_context(tc.tile_pool(name="ob", bufs=3))
        vst_pool = ctx.enter_context(tc.tile_pool(name="vst", bufs=3))

        wq_s = consts.tile([128, DC, 128], BF16, tag="wq_s")
        wk_s = consts.tile([128, DC, 128], BF16, tag="wk_s")
        wv_s = consts.tile([128, DC, 128], BF16, tag="wv_s")
        wp_s = consts.tile([2 * HS, D], BF16, tag="wp_s")

        xts = {}
        for name in ("q", "k", "v"):
            xts[name] = xt_pool.tile(
                [128, DC, N], BF16, tag=f"xt_{name}", name=f"xt_{name}"
            )

        def load_x(eng, name, ms):
            dram = {"q": xqt, "k": xkt, "v": xvt}[name]
            sl = slice(ms * 512, (ms + 1) * 512)
            eng.dma_start(
                out=xts[name][:, :, sl],
                in_=dram[:, sl].rearrange("(c p) n -> p c n", p=128),
            )

        # sync HWDGE queue: wk, xk0, xq0, xq1, xk1 up front; the rest is
        # trickled through the loop (vn transposes interleave).
        # xq1 rides the scalar engine's own HWDGE queue (idle until the
        # first exp) so xk0/xq0/xq1 all transfer in parallel at t=0.
        # All remaining loads are issued up front too: trickled DMA issues
        # would queue behind vn transposes whose semaphore waits block the
        # sync queue for ~10us on HW.
        load_x(nc.scalar, "q", 1)
        nc.sync.dma_start(out=wk_s[:], in_=wk.rearrange("(c p) h -> p c h", p=128))
        load_x(nc.sync, "k", 0)
        load_x(nc.sync, "k", 1)
        load_x(nc.sync, "k", 2)
        load_x(nc.sync, "k", 3)
        load_x(nc.sync, "q", 2)
        load_x(nc.sync, "q", 3)
        sc_eng = [nc.scalar, nc.vector]  # tail ob-copy engines (alternate)

        # gpsimd SWDGE queue: wv, wq, xq0 (parallel to sync's xk0), xv0..3
        nc.gpsimd.dma_start(
            out=wv_s[:], in_=wv.rearrange("(c p) h -> p c h", p=128)
        )
        nc.gpsimd.dma_start(
            out=wq_s[:], in_=wq.rearrange("(c p) h -> p c h", p=128)
        )
        load_x(nc.gpsimd, "q", 0)
        for ms in range(MS):
            load_x(nc.gpsimd, "v", ms)
        nc.gpsimd.dma_start(out=wp_s[:], in_=wp[:])

        # Vn: [m 128, mc, head, 65]; col 64 = ones (rowsum trick)
        vn = consts.tile([128, MC, 2, HS + 1], BF16, tag="vn")
        nc.gpsimd.memset(vn[:, :, :, HS : HS + 1], 1.0)

        wmap = {"q": wq_s, "k": wk_s, "v": wv_s}
        projT = {}
        for name in ("q", "k", "v"):
            projT[name] = proj_pool.tile(
                [128, N], BF16, tag=f"projT_{name}", name=f"projT_{name}"
            )

        # proj psum lives in oB's banks (the h1 accumulator is idle while
        # projections trickle); alternate halves for copy/matmul overlap
        proj_state = {"slot": 0, "cur": None}

        def emit_proj_half(name, ms, half, copy_eng=None):
            # half 0: dc 0,1 into a psum slot; half 1: dc 2,3 + copy out
            sl = slice(ms * 512, (ms + 1) * 512)
            if half == 0:
                s = proj_state["slot"]
                proj_state["slot"] = 1 - s
                proj_state["cur"] = oB[:, s * 512 : (s + 1) * 512]
            ps = proj_state["cur"]
            for dc in (0, 1) if half == 0 else (2, 3):
                nc.tensor.matmul(
                    ps,
                    wmap[name][:, dc, :],
                    xts[name][:, dc, sl],
                    start=(dc == 0),
                    stop=(dc == DC - 1),
                )
            if half == 1:
                if copy_eng is nc.scalar:
                    nc.scalar.copy(projT[name][:, sl], ps)
                else:
                    nc.vector.tensor_copy(projT[name][:, sl], ps)

        def emit_proj_slice(name, ms, copy_eng=None):
            emit_proj_half(name, ms, 0)
            emit_proj_half(name, ms, 1, copy_eng=copy_eng)

        # The xbar transpose corrupts holey/strided destination APs on HW
        # (fine in sim), so transpose into a contiguous staging tile first
        # and let gpsimd scatter it into vn's [mc, h, 65] layout.
        vstage = {}

        def emit_vnq(ms, h):
            # SBUF->SBUF xbar transpose of one (m-slice, head) quarter of VT
            st = vst_pool.tile([128, 4, HS], BF16, tag="vs", name="vs")
            vstage[(ms, h)] = st
            nc.sync.dma_start(
                out=st[:],
                in_=projT["v"][h * HS : (h + 1) * HS, ms * 512 : (ms + 1) * 512],
                transpose=True,
            )

        def emit_vncp(ms, h):
            nc.gpsimd.tensor_copy(
                vn[:, ms * 4 : (ms + 1) * 4, h, 0:HS], vstage.pop((ms, h))[:]
            )

        # PE p-state warmup: the tensor engine only reaches full clock after
        # ~3us of continuous execution; stream dummy matmuls on a zeroed
        # tile while the first input DMAs are in flight so the real
        # projections start at full speed
        # memzero on the scalar engine queue (right behind its xq1 DMA
        # issue): warmup starts ~9us in and bridges into the first real
        # projection without letting the PE clock drop in between
        zwarm = consts.tile([128, 128], BF16, tag="zwarm")
        nc.scalar.memzero(zwarm[:])
        for _ in range(24):
            nc.tensor.matmul(
                sA[:, 0:128], zwarm[:], zwarm[:], start=True, stop=True
            )

        # prologue: only what gates iteration 0 (S needs KT slice 0, QT);
        # q0's psum->sbuf copy runs on the still-idle scalar engine so the
        # two q copies don't serialize on DVE ahead of the first S
        emit_proj_slice("k", 0)
        emit_proj_slice("q", 0, copy_eng=nc.scalar)
        emit_proj_slice("q", 1)

        # deferred trickle queue: each item is (pe_cost, [closures]); each
        # iteration consumes items at its TOP until ~2 matmuls' worth of PE
        # work has been queued (non-PE items — DMA issues, Pool copies,
        # normalize stages — are free), so data is always emitted before the
        # iteration that reads it without over-stuffing the PE stream.
        deferred = [
            (2, [lambda: emit_proj_half("v", 0, 0)]),
            (2, [lambda: emit_proj_half("v", 0, 1)]),
            (0, [lambda: emit_vnq(0, 0)]),
            (0, [lambda: emit_vnq(0, 1)]),
            (2, [lambda: emit_proj_half("k", 1, 0)]),
            (2, [lambda: emit_proj_half("k", 1, 1), lambda: emit_vncp(0, 0)]),
            (2, [lambda: emit_proj_half("v", 1, 0)]),
            (2, [lambda: emit_proj_half("v", 1, 1), lambda: emit_vncp(0, 1)]),
            (0, [lambda: emit_vnq(1, 0)]),
            (2, [lambda: emit_vnq(1, 1), lambda: emit_proj_half("k", 2, 0)]),
            (2, [lambda: emit_proj_half("k", 2, 1)]),
            (2, [lambda: emit_proj_half("v", 2, 0)]),
            (2, [lambda: emit_proj_half("v", 2, 1), lambda: emit_vncp(1, 0)]),
            (0, [lambda: emit_vnq(2, 0), lambda: emit_vncp(1, 1)]),
            (2, [lambda: emit_vnq(2, 1), lambda: emit_proj_half("k", 3, 0)]),
            (2, [lambda: emit_proj_half("k", 3, 1), lambda: emit_vncp(2, 0)]),
            (2, [lambda: emit_proj_half("v", 3, 0), lambda: emit_vncp(2, 1)]),
            (2, [lambda: emit_proj_half("v", 3, 1)]),
            (0, [lambda: emit_vnq(3, 0)]),
            (2, [lambda: emit_vnq(3, 1), lambda: emit_proj_half("q", 2, 0)]),
            (2, [lambda: emit_proj_half("q", 2, 1), lambda: emit_vncp(3, 0)]),
            (2, [lambda: emit_proj_half("q", 3, 0), lambda: emit_vncp(3, 1)]),
            (2, [lambda: emit_proj_half("q", 3, 1)]),
        ]

        # normalization chain, split into trickle-able stages.
        # rr = 1/rowsum (DVE, psum row 64 -> sbuf fp32 at partition 0)
        # rb = broadcast to 64 partitions (gpsimd)
        # un = O^T[0:64] * rb (DVE, psum x sbuf -> sbuf bf16)
        def emit_recip(o_ps_, hh, q0, q1):
            # reciprocal_approx_fast corrupts on HW when its input is a PSUM
            # row off partition 0; copy the rowsum row down first (DVE
            # partition-shift on a plain TensorCopy is exact)
            r0 = rr_pool.tile([1, QW], FP32, tag="r0", name="r0")
            nc.vector.tensor_copy(r0[0:1, q0:q1], o_ps_[hh][HS : HS + 1, q0:q1])
            rr = rr_pool.tile([1, QW], FP32, tag="rr", name="rr")
            nc.vector.reciprocal_approx_fast(rr[0:1, q0:q1], r0[0:1, q0:q1])
            return rr

        def emit_bcast(rr, q0, q1):
            rb = rb_pool.tile([HS, QW], FP32, tag="rb", name="rb")
            nc.gpsimd.partition_broadcast(rb[:, q0:q1], rr[0:1, q0:q1])
            return rb

        def emit_mul(o_ps_, rb, un2_, hh, q0, q1, eng=None):
            (eng or nc.vector).tensor_mul(
                un2_[HS * hh : HS * hh + HS, q0:q1],
                o_ps_[hh][0:HS, q0:q1],
                rb[:, q0:q1],
            )

        def emit_final(qh_, un2_, c, f_ps, ob_eng=None):
            nc.tensor.matmul(
                f_ps,
                un2_[:, c * 128 : (c + 1) * 128],
                wp_s[:],
                start=True,
                stop=True,
            )
            ob = ob_pool.tile([128, D], BF16, tag="ob", name="ob")
            if ob_eng is nc.scalar:
                nc.scalar.copy(ob[:], f_ps)
            else:
                (ob_eng or nc.vector).tensor_copy(ob[:], f_ps)
            nc.sync.dma_start(
                out=out[qh_ * QW + c * 128 : qh_ * QW + (c + 1) * 128, :],
                in_=ob[:],
            )

        # attention loop: heads sequential within each q-half so h0's
        # normalization trickles during h1's m-loop; leftover PVs at a qh
        # boundary drain into the next qh's loop instead of bursting
        qt2, kt2 = projT["q"], projT["k"]
        o_ps = {0: oA, 1: oB}
        s_it = [0]  # global S iteration counter for sA/sB alternation
        pend = []

        def pv(hh, j, p_sb):
            for sl in range(QW // 512):
                nc.tensor.matmul(
                    o_ps[hh][0 : HS + 1, sl * 512 : (sl + 1) * 512],
                    vn[:, j, hh, :],
                    p_sb[:, sl * 512 : (sl + 1) * 512],
                    start=(j == 0),
                    stop=(j == MC - 1),
                )

        def chain_items(hh, un2_):
            # normalization chain for one completed head, as trickle items
            box = {}
            return [
                (0, [lambda: box.__setitem__("rr", emit_recip(o_ps, hh, 0, QW))]),
                (0, [lambda: box.__setitem__("rb", emit_bcast(box["rr"], 0, QW))]),
                (0, [lambda: emit_mul(o_ps, box["rb"], un2_, hh, 0, QW)]),
            ]

        def pv_pop():
            qh_, hh_, mc_, p_, un2_ = pend.pop(0)
            pv(hh_, mc_, p_)
            if mc_ == MC - 1:
                if hh_ == 0:
                    deferred.extend(chain_items(0, un2_))
                elif qh_ < QH - 1:
                    deferred.extend(chain_items(1, un2_))
                    for c in range(QW // 128):
                        # f_ps alternates oB halves (h1 accumulator idle)
                        deferred.append(
                            (
                                2,
                                [
                                    lambda un2__=un2_, c_=c, qh__=qh_: emit_final(
                                        qh__,
                                        un2__,
                                        c_,
                                        oB[
                                            :,
                                            (c_ % 2) * 512 : (c_ % 2) * 512
                                            + 512,
                                        ],
                                    )
                                ],
                            )
                        )

        for qh in range(QH):
            un2 = un_pool.tile([128, QW], BF16, tag="un")
            for hh in range(2):
                for mc in range(MC):
                    it = hh * MC + mc
                    budget = 2 if (qh == 0 and it < 12) else 1
                    nb = 0
                    while deferred and nb < budget:
                        for fn in deferred.pop(0)[1]:
                            fn()
                        nb += 1
                    hs0 = HS * hh
                    s_ps = (sA, sB)[s_it[0] % 2]
                    s_it[0] += 1
                    for sl in range(QW // 512):
                        nc.tensor.matmul(
                            s_ps[:, sl * 512 : (sl + 1) * 512],
                            kt2[hs0 : hs0 + HS, mc * 128 : (mc + 1) * 128],
                            qt2[
                                hs0 : hs0 + HS,
                                qh * QW + sl * 512 : qh * QW + (sl + 1) * 512,
                            ],
                            start=True,
                            stop=True,
                        )
                    p_sb = pt_pool.tile([128, QW], BF16, tag="p", name="p_sb")
                    last = qh == QH - 1 and it == 2 * MC - 1
                    if last:
                        # split the final exp so PV / normalize / outproj of
                        # the first q-half start half an exp earlier
                        for sl in range(2):
                            nc.scalar.activation(
                                p_sb[:, sl * 512 : (sl + 1) * 512],
                                s_ps[:, sl * 512 : (sl + 1) * 512],
                                mybir.ActivationFunctionType.Exp,
                            )
                    else:
                        nc.scalar.activation(
                            p_sb[:], s_ps[:], mybir.ActivationFunctionType.Exp
                        )
                    pend.append((qh, hh, mc, p_sb, un2))
                    # qh0: lag 8 (vn staging arrives ~iter 10); qh1: lag 5
                    # so the previous qh's leftovers (and with them the
                    # trickled outproj into oB) clear before PV(h1) reopens oB
                    lag = (8 if qh == 0 else 5) if not last else 1
                    nd = 0
                    while len(pend) > lag and nd < 2:
                        pv_pop()
                        nd += 1
        while pend:
            pv_pop()
        # flush any remaining trickle items (qh1 h0-chain may still be queued)
        while deferred:
            for fn in deferred.pop(0)[1]:
                fn()

        # tail: last head's chain split by q-halves; outproj pipelined
        # behind on four f_ps slots in the freed S banks, ob copies
        # alternating scalar/vector (both idle by now)
        rr0 = emit_recip(o_ps, 1, 0, 512)
        rb0 = emit_bcast(rr0, 0, 512)
        emit_mul(o_ps, rb0, un2, 1, 0, 512)
        rr1 = emit_recip(o_ps, 1, 512, QW)
        rb1 = emit_bcast(rr1, 512, QW)
        emit_mul(o_ps, rb1, un2, 1, 512, QW)
        f_slots = [sA[:, 0:512], sA[:, 512:1024], sB[:, 0:512], sB[:, 512:1024]]
        for c in range(QW // 128):
            emit_final(QH - 1, un2, c, f_slots[c % 4], ob_eng=sc_eng[c % 2])
    if finalize:
        nc.finalize()
    return nc


_NC_CACHE = None


def _get_nc():
    global _NC_CACHE
    if _NC_CACHE is None:
        _NC_CACHE = build_nc()
    return _NC_CACHE


def make_in_maps(inputs):
    query = np.asarray(inputs["query"], np.float32)
    key = np.asarray(inputs["key"], np.float32)
    value = np.asarray(inputs["value"], np.float32)
    Wq = np.asarray(inputs["Wq"], np.float32) / np.sqrt(np.float32(HS))
    Wk = np.asarray(inputs["Wk"], np.float32)
    Wv = np.asarray(inputs["Wv"], np.float32)
    Wp = np.asarray(inputs["Wp"], np.float32)

    in_maps = []
    for c in range(NCORES):
        b = c // 4
        h0 = 2 * (c % 4)
        in_maps.append(
            {
                "xqt": np.ascontiguousarray(query[b].T).astype(nbf16),
                "xkt": np.ascontiguousarray(key[b].T).astype(nbf16),
                "xvt": np.ascontiguousarray(value[b].T).astype(nbf16),
                "wq": np.concatenate([Wq[h0], Wq[h0 + 1]], axis=1).astype(nbf16),
                "wk": np.concatenate([Wk[h0], Wk[h0 + 1]], axis=1).astype(nbf16),
                "wv": np.concatenate([Wv[h0], Wv[h0 + 1]], axis=1).astype(nbf16),
                "wp": np.concatenate([Wp[h0], Wp[h0 + 1]], axis=0).astype(nbf16),
            }
        )
    return in_maps


def kernel(query, key, value, Wq, Wk, Wv, Wp):
    in_maps = make_in_maps(
        dict(query=query, key=key, value=value, Wq=Wq, Wk=Wk, Wv=Wv, Wp=Wp)
    )
    nc = _get_nc()
    res = run_bass_kernel_spmd(nc, in_maps, list(range(NCORES)))
    out = np.zeros((B, N, D), np.float32)
    for c in range(NCORES):
        out[c // 4] += np.asarray(res.results[c]["out"], np.float32)
    return out


if __name__ == "__main__":
    d = np.load("/root/problem/work/ref.npz")
    got = kernel(
        d["query"], d["key"], d["value"], d["Wq"], d["Wk"], d["Wv"], d["Wp"]
    )
    exp = d["expected"]
    rel = np.linalg.norm(got - exp) / np.linalg.norm(exp)
    print("Relative error:", rel)
